# revision 1
# baseline (speedup 1.0000x reference)
"""Trainium2 Bass kernel for nn_NeuronCircuit_45140106281445 (MoE-routed attention).

8-core SPMD plan:
  - Rank-sharded compress: core c owns rank-columns [64c, 64c+64) of the shared
    compress neuron bank and computes its Q/K/V rank slice for ALL 2048 tokens
    densely over all 32 experts (the projection is shared across the Q/K/V
    routers), then top-8 gated-combines on the vector engine.
  - Attention: core c's rank slice is exactly head c, so attention for head c
    (both batches) runs with zero communication.
  - Expand-router scores: computed token-major per chunk (lhsT = local attn_out
    slice, rhs = Wo slice) and AllReduce-summed across cores; lands directly in
    the [token, expert] layout the gating needs (no transposes).
  - attn_out is AllGathered in fp16 (values only -- routing uses the exact fp32
    scores), halving collective bytes.
  - d_model-sharded expand: core c owns output columns [128c, 128c+128), dense
    over all 32 experts in fp16 (post-routing, so ~2^-11 operand rounding only
    perturbs output values, not expert selection), top-4 gated combine.
  - Phase C runs per batch: batch 0's expand overlaps batch 1's collectives.

Precision: top-k selection flips are the dominant error mode and need
~1e-6-level accuracy on everything feeding a router (a single expand-router
flip costs ~5e-2 max-rel error vs the fp32 reference -- measured in float64
simulation).  So compress scores and the compress main matmul use a manual
fp16 hi/lo split (3 fp16 passes; products are exact into the fp32 PSUM
accumulator, so accuracy is ~fp32 at 3/4 the cost of the HW fp32 4-pass
path), and attention + expand router scores use true-fp32 matmuls.  The
expand main matmul is post-routing and linear in the output, so fp16 is safe.
"""

from contextlib import ExitStack

import numpy as np

import concourse.bass as bass  # noqa: F401
import concourse.mybir as mybir
import concourse.tile as tile
from concourse import bacc
from concourse.bass_utils import run_bass_kernel_spmd

F32 = mybir.dt.float32
F16 = mybir.dt.float16
AX = mybir.AxisListType
OP = mybir.AluOpType
AF = mybir.ActivationFunctionType

N_CORES = 8
B, S, D, R, H, DH = 2, 1024, 1024, 512, 8, 64
BS = B * S  # 2048 tokens
NEXP = 32
TCH = BS // 128  # 16 token chunks
KD = D // 128  # 8 k-tiles over d_model
KR = R // 128  # 4 k-tiles over rank
NQ = S // 128  # 8 query chunks per batch
NEG = -1e30

# Tuning flags (fallbacks if a mechanism misbehaves on HW).
UNSTAB_EXP = True  # skip softmax max-subtraction (|logit| << 88 verified)
DMA_ACCUM = False  # SWDGE accumulate-DMA does not accumulate on this stack
GPSIMD_TREE = True  # expand combine tree split across DVE + gpsimd


def _build_program():
    nc = bacc.Bacc(
        "TRN2", target_bir_lowering=False, debug=False, num_devices=N_CORES
    )
    io = dict(
        xth=nc.dram_tensor("xth", [D, BS], F16, kind="ExternalInput"),
        xtl=nc.dram_tensor("xtl", [D, BS], F16, kind="ExternalInput"),
        cwh=nc.dram_tensor("cwh", [128, KD, NEXP * DH], F16, kind="ExternalInput"),
        cwl=nc.dram_tensor("cwl", [128, KD, NEXP * DH], F16, kind="ExternalInput"),
        ew=nc.dram_tensor("ew", [128, KR, NEXP * 128], F16, kind="ExternalInput"),
        wrh=nc.dram_tensor("wrh", [128, KD, 96], F16, kind="ExternalInput"),
        wrl=nc.dram_tensor("wrl", [128, KD, 96], F16, kind="ExternalInput"),
        wol=nc.dram_tensor("wol", [64, 32], F32, kind="ExternalInput"),
        ident=nc.dram_tensor("ident", [128, 128], F32, kind="ExternalInput"),
        causal=nc.dram_tensor("causal", [128, 128], F32, kind="ExternalInput"),
        outt=nc.dram_tensor("outt", [TCH, 128, 128], F32, kind="ExternalOutput"),
    )
    with tile.TileContext(nc) as tc:
        _emit(nc, tc, io)
    nc.compile()
    return nc


def _emit_gating(nc, pool, SC, GATES, nrow, t8_tag, topk):
    """Top-k-of-32 softmax gating over SC [128, nrow, 32] -> GATES (same).

    Uses max8 to get the top-8 values per row; for topk<8 the unused entries
    are overwritten with 1e30 so match_replace8 only knocks out the true
    top-k.  GATES = softmax over selected entries, 0 elsewhere.
    """
    T8 = pool.tile([128, nrow * 8], F32, tag=t8_tag, name=f"{t8_tag}")
    WORK = pool.tile([128, nrow, 32], F32, tag=f"{t8_tag}_wk")
    SCF = SC.rearrange("p c n -> p (c n)")
    for j in range(nrow):
        nc.vector.max(T8[:, j * 8 : j * 8 + 8], SCF[:, j * 32 : (j + 1) * 32])
    if topk < 8:
        T8V = T8[:].rearrange("p (j e) -> p j e", e=8)
        nc.vector.memset(T8V[:, :, topk:8], 1e30)
    for j in range(nrow):
        nc.vector.match_replace(
            WORK[:, j, :],
            in_to_replace=T8[:, j * 8 : j * 8 + 8],
            in_values=SCF[:, j * 32 : (j + 1) * 32],
            imm_value=NEG,
        )
    WKF = WORK[:].rearrange("p j n -> p (j n)")
    # sel mask in-place into WORK: 1.0 at top-k positions, 0 elsewhere
    nc.vector.tensor_sub(WKF, SCF, WKF)
    nc.vector.tensor_scalar_min(WKF, WKF, 1.0)
    M1 = T8[:].rearrange("p (j e) -> p j e", e=8)[:, :, 0:1]
    GF = GATES.rearrange("p j n -> p (j n)")
    nc.vector.tensor_tensor(
        GATES, SC, M1.to_broadcast([128, nrow, 32]), op=OP.subtract
    )
    nc.scalar.activation(GF, GF, AF.Exp)
    nc.vector.tensor_mul(GF, GF, WKF)
    Z = pool.tile([128, nrow], F32, tag=f"{t8_tag}_z")
    nc.vector.tensor_reduce(Z[:], GATES, axis=AX.X, op=OP.add)
    RZ = pool.tile([128, nrow], F32, tag=f"{t8_tag}_rz")
    nc.vector.reciprocal(RZ[:], Z[:])
    nc.vector.tensor_tensor(
        GATES,
        GATES,
        RZ[:, :, None].to_broadcast([128, nrow, 32]),
        op=OP.mult,
    )


def _emit(nc, tc, io):
    with ExitStack() as ctx:
        glob = ctx.enter_context(tc.tile_pool(name="glob", bufs=1))
        dr = ctx.enter_context(tc.tile_pool(name="dram", bufs=1, space="DRAM"))

        IDENT = glob.tile([128, 128], F32, tag="ident")
        nc.sync.dma_start(IDENT[:], io["ident"][:])
        CAUSAL = glob.tile([128, 128], F32, tag="causal")
        nc.sync.dma_start(CAUSAL[:], io["causal"][:])
        ACC = {
            p: glob.tile([128, TCH, DH], F32, tag=f"acc_{p}", name=f"acc_{p}")
            for p in "qkv"
        }
        OUT = glob.tile([128, TCH, 128], F32, tag="out")
        if DMA_ACCUM:
            nc.gpsimd.memset(OUT[:].rearrange("p c r -> p (c r)"), 0.0)

        # ================= Phase A: scores + gating + compress =================
        with (
            tc.tile_pool(name="pa", bufs=1) as pa,
            tc.tile_pool(name="pa_s", bufs=2) as pas,
            tc.tile_pool(name="psA", bufs=2, space="PSUM") as psA,
        ):
            # load order matters: router weights + X first (scores path),
            # neuron banks afterwards (needed ~100us later)
            WRH = pa.tile([128, KD, 96], F16, tag="wrh")
            nc.sync.dma_start(WRH[:], io["wrh"][:])
            WRL = pa.tile([128, KD, 96], F16, tag="wrl")
            nc.sync.dma_start(WRL[:], io["wrl"][:])
            XTH = pa.tile([128, KD, BS], F16, tag="xth")
            XTL = pa.tile([128, KD, BS], F16, tag="xtl")
            for kt in range(KD):  # per-ktile DMAs so matmuls start early
                nc.sync.dma_start(
                    XTH[:, kt, :], io["xth"][kt * 128 : (kt + 1) * 128, :]
                )
                nc.sync.dma_start(
                    XTL[:, kt, :], io["xtl"][kt * 128 : (kt + 1) * 128, :]
                )
            CWH = pa.tile([128, KD, NEXP * DH], F16, tag="cwh")
            CWL = pa.tile([128, KD, NEXP * DH], F16, tag="cwl")
            for kt in range(KD):
                nc.sync.dma_start(CWH[:, kt, :], io["cwh"][:, kt, :])
                nc.sync.dma_start(CWL[:, kt, :], io["cwl"][:, kt, :])

            # ---- compress router scores (fp16-split), scoresT [96, 2048] ----
            ps_sc = psA.tile([128, BS], F32, tag="big")
            for kt in range(KD):
                terms = ((WRH, XTH), (WRH, XTL), (WRL, XTH))
                for ti, (wt, xt_) in enumerate(terms):
                    for nch in range(4):
                        nc.tensor.matmul(
                            ps_sc[:96, nch * 512 : (nch + 1) * 512],
                            lhsT=wt[:, kt, :],
                            rhs=xt_[:, kt, nch * 512 : (nch + 1) * 512],
                            start=(kt == 0 and ti == 0),
                            stop=(kt == KD - 1 and ti == 2),
                        )
            ST = pa.tile([96, BS], F32, tag="scoresT")
            nc.scalar.copy(ST[:], ps_sc[:96, :])
            # transpose to SCORES [128, TCH, 96] (chunk-major: q|k|v per chunk)
            SCORES = pa.tile([128, TCH, 96], F32, tag="scores")
            for i in range(TCH):
                pt = psA.tile([128, 128], F32, tag="big")
                nc.tensor.transpose(
                    pt[:, :96], ST[:, i * 128 : (i + 1) * 128], IDENT[:96, :96]
                )
                nc.any.tensor_copy(SCORES[:, i, :], pt[:, :96])

            # ---- gating: top-8 of 32 for q/k/v ----
            NROW = TCH * 3  # 48 rows of 32 scores; row j = chunk*3 + proj
            GATES = pa.tile([128, NROW, 32], F32, tag="gates")
            _emit_gating(nc, pa, SCORES[:].rearrange("p c (x n) -> p (c x) n", n=32),
                         GATES[:], NROW, "t8c", 8)

            # ---- compress main (fp16-split) + gated combine ----
            for i in range(TCH):
                ps_p = psA.tile([128, NEXP * DH], F32, tag="big")
                for kt in range(KD):
                    tsl = slice(i * 128, (i + 1) * 128)
                    terms = ((XTH, CWH), (XTH, CWL), (XTL, CWH))
                    for ti, (xt_, cw_) in enumerate(terms):
                        for g in range(4):
                            nc.tensor.matmul(
                                ps_p[:, g * 512 : (g + 1) * 512],
                                lhsT=xt_[:, kt, tsl],
                                rhs=cw_[:, kt, g * 512 : (g + 1) * 512],
                                start=(kt == 0 and ti == 0),
                                stop=(kt == KD - 1 and ti == 2),
                            )
                psv = ps_p[:].rearrange("p (n r) -> p n r", r=DH)
                for pi, p in enumerate("qkv"):
                    stg = pas.tile([128, NEXP * DH], F32, tag="stage_c")
                    gv = GATES[:, i * 3 + pi, :, None]
                    nc.vector.tensor_tensor(
                        stg[:].rearrange("p (n r) -> p n r", r=DH),
                        psv,
                        gv.to_broadcast([128, NEXP, DH]),
                        op=OP.mult,
                    )
                    w = NEXP * DH  # 2048
                    while w > 2 * DH:
                        nc.vector.tensor_add(
                            stg[:, : w // 2], stg[:, : w // 2], stg[:, w // 2 : w]
                        )
                        w //= 2
                    nc.vector.tensor_add(
                        ACC[p][:, i, :], stg[:, :DH], stg[:, DH : 2 * DH]
                    )

        # ================= Phase B: attention (head = core id) =================
        pc = ctx.enter_context(tc.tile_pool(name="pc", bufs=1))
        EW = pc.tile([128, KR, NEXP * 128], F16, tag="ew")
        nc.sync.dma_start(EW[:], io["ew"][:])  # prefetch for phase C
        ATF = pc.tile([128, KR, BS], F16, tag="attnT_full")
        SCO = pc.tile([128, TCH, 32], F32, tag="sco")

        with (
            tc.tile_pool(name="pb", bufs=1) as pb,
            tc.tile_pool(name="pb_s", bufs=3) as pbs,
            tc.tile_pool(name="psB1", bufs=2, space="PSUM") as psB1,
            tc.tile_pool(name="psB2", bufs=1, space="PSUM") as psB2,
        ):
            QT = pb.tile([64, BS], F32, tag="qt")
            KT = pb.tile([64, BS], F32, tag="kt")
            for name, dst in (("q", QT), ("k", KT)):
                for i in range(TCH):
                    pt = psB1.tile([128, 128], F32, tag="tr")
                    nc.tensor.transpose(pt[:64, :], ACC[name][:, i, :], IDENT[:])
                    nc.any.tensor_copy(dst[:, i * 128 : (i + 1) * 128], pt[:64, :])

            WOL = pb.tile([64, 32], F32, tag="wol")
            nc.sync.dma_start(WOL[:], io["wol"][:])
            ATL = pb.tile([64, BS], F32, tag="attnT_local")
            ATL16 = pb.tile([64, BS], F16, tag="attnT_local16")
            ATS = [
                pb.tile([128, S], F32, tag=f"ats_{ki}", name=f"ats_{ki}")
                for ki in range(NQ)
            ]

            for b in range(B):
                off = b * S
                for qi in range(NQ):
                    W = 128 * (qi + 1)
                    psA_t = psB1.tile([128, S], F32, tag="attn")
                    for ncb in range((W + 511) // 512):
                        lo, hi = ncb * 512, min(W, ncb * 512 + 512)
                        nc.tensor.matmul(
                            psA_t[:, lo:hi],
                            lhsT=QT[:, off + qi * 128 : off + (qi + 1) * 128],
                            rhs=KT[:, off + lo : off + hi],
                            start=True,
                            stop=True,
                        )
                    nc.vector.tensor_add(
                        psA_t[:, qi * 128 : W], psA_t[:, qi * 128 : W], CAUSAL[:]
                    )
                    Ab = pbs.tile([128, S], F32, tag="abuf")
                    zr = pbs.tile([128, 1], F32, tag="zrow")
                    if UNSTAB_EXP:
                        nc.scalar.activation(
                            Ab[:, :W],
                            psA_t[:, :W],
                            AF.Exp,
                            scale=0.125,
                            accum_out=zr[:],
                        )
                    else:
                        mx = pbs.tile([128, 1], F32, tag="mx")
                        nc.vector.tensor_reduce(
                            mx[:], psA_t[:, :W], axis=AX.X, op=OP.max
                        )
                        negm = pbs.tile([128, 1], F32, tag="negm")
                        nc.vector.tensor_scalar_mul(negm[:], mx[:], -0.125)
                        nc.scalar.activation(
                            Ab[:, :W],
                            psA_t[:, :W],
                            AF.Exp,
                            bias=negm[:],
                            scale=0.125,
                            accum_out=zr[:],
                        )
                    rz = pbs.tile([128, 1], F32, tag="rzrow")
                    nc.vector.reciprocal(rz[:], zr[:])
                    nc.vector.tensor_scalar_mul(Ab[:, :W], Ab[:, :W], rz[:])
                    for ki in range(qi + 1):
                        ptA = psB1.tile([128, 128], F32, tag="tr")
                        nc.tensor.transpose(
                            ptA[:], Ab[:, ki * 128 : (ki + 1) * 128], IDENT[:]
                        )
                        nc.any.tensor_copy(
                            ATS[ki][:, qi * 128 : (qi + 1) * 128], ptA[:]
                        )
                # AV, causal-truncated: ATS[ki] only has valid (nonzero) data in
                # columns >= ki*128, so clip each contribution to its live range.
                psO = psB2.tile([64, S], F32, tag="attno")
                for ncb in range(2):
                    lo, hi = ncb * 512, (ncb + 1) * 512
                    kis = [ki for ki in range(NQ) if ki * 128 < hi]
                    for ki in kis:
                        c0 = max(lo, ki * 128)
                        nc.tensor.matmul(
                            psO[:, c0:hi],
                            lhsT=ACC["v"][:, b * NQ + ki, :],
                            rhs=ATS[ki][:, c0:hi],
                            start=(ki == 0),
                            stop=(ki == kis[-1]),
                        )
                nc.scalar.copy(ATL[:, off : off + S], psO[:])

                # expand-router partial scores, token-major: [128 tok, 32] per
                # chunk via lhsT = local attn_out chunk (exact fp32).
                ps_q = psB1.tile([128, NQ * 32], F32, tag="attn", name=f"ps_q{b}")
                for c in range(NQ):
                    nc.tensor.matmul(
                        ps_q[:, c * 32 : (c + 1) * 32],
                        lhsT=ATL[:, off + c * 128 : off + (c + 1) * 128],
                        rhs=WOL[:],
                        start=True,
                        stop=True,
                    )
                SOP = pbs.tile([128, NQ * 32], F32, tag="so_part")
                nc.any.tensor_copy(SOP[:], ps_q[:])

                # per-batch collectives: AllReduce (small, first) then AllGather.
                # batch 0's collectives hide under batch 1's attention; batch 1's
                # hide under batch 0's expand (phase C runs per batch).
                bi_ar = dr.tile([128, NQ * 32], F32, name=f"bi_ar{b}")
                bo_ar = dr.tile(
                    [128, NQ * 32], F32, addr_space="Shared", name=f"bo_ar{b}"
                )
                nc.sync.dma_start(bi_ar[:], SOP[:])
                nc.gpsimd.collective_compute(
                    "AllReduce",
                    OP.add,
                    replica_groups=[list(range(N_CORES))],
                    ins=[bi_ar[:]],
                    outs=[bo_ar[:]],
                )
                nc.any.tensor_copy(ATL16[:, off : off + S], ATL[:, off : off + S])
                bi_ag = dr.tile([64, S], F16, name=f"bi_ag{b}")
                bo_ag = dr.tile(
                    [N_CORES * 64, S], F16, addr_space="Shared", name=f"bo_ag{b}"
                )
                nc.sync.dma_start(bi_ag[:], ATL16[:, off : off + S])
                nc.gpsimd.collective_compute(
                    "AllGather",
                    OP.bypass,
                    replica_groups=[list(range(N_CORES))],
                    ins=[bi_ag[:]],
                    outs=[bo_ag[:]],
                )
                # land this batch's halves
                nc.sync.dma_start(
                    SCO[:, b * NQ : (b + 1) * NQ, :],
                    bo_ar[:].rearrange("p (c n) -> p c n", n=32),
                )
                nc.sync.dma_start(
                    ATF[:, :, off : off + S],
                    bo_ag[:].rearrange("(k p) t -> p k t", p=128),
                )

        # ================= Phase C: gating + expand, per batch =================
        with (
            tc.tile_pool(name="pd", bufs=1) as pd,
            tc.tile_pool(name="pc_s", bufs=2) as pcs,
            tc.tile_pool(name="psC", bufs=2, space="PSUM") as psC,
        ):
            GO = pd.tile([128, TCH, 32], F32, tag="go")
            for b in range(B):
                # ---- gating: top-4 of 32 (scores already token-major) ----
                _emit_gating(
                    nc, pd,
                    SCO[:, b * NQ : (b + 1) * NQ, :],
                    GO[:, b * NQ : (b + 1) * NQ, :],
                    NQ, f"t8o{b}", 4,
                )

                # ---- expand main (fp16) + top-4 combine ----
                for i in range(b * NQ, (b + 1) * NQ):
                    for h in range(2):
                        ps_e = psC.tile([128, 2048], F32, tag="big")
                        for kt in range(KR):
                            for g in range(4):
                                nc.tensor.matmul(
                                    ps_e[:, g * 512 : (g + 1) * 512],
                                    lhsT=ATF[:, kt, i * 128 : (i + 1) * 128],
                                    rhs=EW[
                                        :,
                                        kt,
                                        h * 2048 + g * 512 : h * 2048 + (g + 1) * 512,
                                    ],
                                    start=(kt == 0),
                                    stop=(kt == KR - 1),
                                )
                        stg = pcs.tile([128, 2048], F32, tag="stage_e")
                        gv = GO[:, i, h * 16 : (h + 1) * 16, None]
                        nc.vector.tensor_tensor(
                            stg[:].rearrange("p (n r) -> p n r", r=128),
                            ps_e[:].rearrange("p (n r) -> p n r", r=128),
                            gv.to_broadcast([128, 16, 128]),
                            op=OP.mult,
                        )
                        if DMA_ACCUM:
                            nc.vector.tensor_add(
                                stg[:, :1024], stg[:, :1024], stg[:, 1024:]
                            )
                            nc.gpsimd.dma_start(
                                OUT[:, i : i + 1, :].to_broadcast([128, 8, 128]),
                                stg[:, :1024].rearrange("p (n r) -> p n r", r=128),
                                accum_op=OP.add,
                            )
                        elif GPSIMD_TREE:
                            # split the reduction across gpsimd + DVE so the
                            # PSUM-evacuating multiply (DVE-only) rate-matches
                            # the tensor engine.
                            nc.gpsimd.tensor_add(
                                stg[:, :1024], stg[:, :1024], stg[:, 1024:]
                            )
                            nc.gpsimd.tensor_add(
                                stg[:, :512], stg[:, :512], stg[:, 512:1024]
                            )
                            nc.vector.tensor_add(
                                stg[:, :256], stg[:, :256], stg[:, 256:512]
                            )
                            if h == 0:
                                nc.vector.tensor_add(
                                    OUT[:, i, :], stg[:, :128], stg[:, 128:256]
                                )
                            else:
                                tmp = pcs.tile([128, 128], F32, tag="tmp_e")
                                nc.vector.tensor_add(
                                    tmp[:], stg[:, :128], stg[:, 128:256]
                                )
                                nc.vector.tensor_add(
                                    OUT[:, i, :], OUT[:, i, :], tmp[:]
                                )
                        else:
                            w = 2048
                            while w > 256:
                                nc.vector.tensor_add(
                                    stg[:, : w // 2],
                                    stg[:, : w // 2],
                                    stg[:, w // 2 : w],
                                )
                                w //= 2
                            if h == 0:
                                nc.vector.tensor_add(
                                    OUT[:, i, :], stg[:, :128], stg[:, 128:256]
                                )
                            else:
                                tmp = pcs.tile([128, 128], F32, tag="tmp_e")
                                nc.vector.tensor_add(
                                    tmp[:], stg[:, :128], stg[:, 128:256]
                                )
                                nc.vector.tensor_add(
                                    OUT[:, i, :], OUT[:, i, :], tmp[:]
                                )
                    nc.sync.dma_start(io["outt"][i], OUT[:, i, :])


_PROGRAM = None


def _get_program():
    global _PROGRAM
    if _PROGRAM is None:
        _PROGRAM = _build_program()
    return _PROGRAM


def _hilo(a32):
    """fp16 hi/lo split: a32 ~= hi + lo with the product path exact in FP22."""
    hi = a32.astype(np.float16)
    lo = (a32 - hi.astype(np.float32)).astype(np.float16)
    return np.ascontiguousarray(hi), np.ascontiguousarray(lo)


def _prep_inputs(x, compress_neurons, expand_neurons, Wq, Wk, Wv, Wo):
    """Build the 8 per-core input maps (numpy, DMA-friendly layouts)."""
    X = np.ascontiguousarray(x.reshape(BS, D), dtype=np.float32)
    xt = np.ascontiguousarray(X.T)  # [D, BS]
    xth, xtl = _hilo(xt)
    wr = (
        np.stack([Wq, Wk, Wv], axis=0)  # [3, 32, D]
        .transpose(2, 0, 1)  # [D, 3, 32]
        .reshape(D, 96)
        .reshape(KD, 128, 96)
        .transpose(1, 0, 2)  # [128, KD, 96]
    )
    wr = np.ascontiguousarray(wr, dtype=np.float32)
    wrh, wrl = _hilo(wr)
    ident = np.eye(128, dtype=np.float32)
    causal = np.where(
        np.arange(128)[None, :] <= np.arange(128)[:, None], 0.0, NEG
    ).astype(np.float32)

    in_maps = []
    for c in range(N_CORES):
        cwc = compress_neurons[:, :, c * DH : (c + 1) * DH]  # [32, D, 64]
        cw = np.ascontiguousarray(
            cwc.reshape(NEXP, KD, 128, DH)
            .transpose(2, 1, 0, 3)  # [128, KD, 32, 64]
            .reshape(128, KD, NEXP * DH),
            dtype=np.float32,
        )
        cwh, cwl = _hilo(cw)
        ewc = expand_neurons[:, :, c * 128 : (c + 1) * 128]  # [32, R, 128]
        ew = np.ascontiguousarray(
            ewc.reshape(NEXP, KR, 128, 128)
            .transpose(2, 1, 0, 3)  # [128, KR, 32, 128]
            .reshape(128, KR, NEXP * 128),
            dtype=np.float16,
        )
        wol = np.ascontiguousarray(Wo[:, c * DH : (c + 1) * DH].T, dtype=np.float32)
        in_maps.append(
            dict(
                xth=xth,
                xtl=xtl,
                cwh=cwh,
                cwl=cwl,
                ew=ew,
                wrh=wrh,
                wrl=wrl,
                wol=wol,
                ident=ident,
                causal=causal,
            )
        )
    return in_maps


def kernel(x, mask, compress_neurons, expand_neurons, Wq, Wk, Wv, Wo):
    """Full-input entry point; returns the [B, S, D] fp32 output."""
    x = np.asarray(x, dtype=np.float32)
    compress_neurons = np.asarray(compress_neurons, dtype=np.float32)
    expand_neurons = np.asarray(expand_neurons, dtype=np.float32)
    Wq, Wk, Wv, Wo = (np.asarray(w, dtype=np.float32) for w in (Wq, Wk, Wv, Wo))

    nc = _get_program()
    in_maps = _prep_inputs(x, compress_neurons, expand_neurons, Wq, Wk, Wv, Wo)
    res = run_bass_kernel_spmd(nc, in_maps, core_ids=list(range(N_CORES)))
    out = np.empty((BS, D), dtype=np.float32)
    for c in range(N_CORES):
        oc = res.results[c]["outt"]  # [TCH, 128, 128]
        out[:, c * 128 : (c + 1) * 128] = oc.reshape(BS, 128)
    return out.reshape(B, S, D)

